# revision 34
# baseline (speedup 1.0000x reference)
"""Multi-head attention (S=2048, B=2, D=1024, H=16) on 8 Trainium2 NeuronCores.

Sharding: batch x heads. Core c handles batch c//4 and heads (c%4)*4..+4,
processed as two head-pairs that map onto a pipelined attention loop
(scores row-tiled per head pair, softmax denominator via a ones-column in V,
QKV projections restricted to the core's 256 output dims, row-parallel
output projection accumulated over both pairs in PSUM). The host sums the
4 partial outputs per batch and adds bo.

On-device compute is fp16 with fp32 PSUM accumulation; output partials are
written fp16. x loads stream in column-chunk order across 4 DMA queues so
the first scores matmul can issue ~17us into the run.
"""

import math

import numpy as np

S, B, D, H = 2048, 2, 1024, 16
DK = D // H               # 64
NCORES = 8
HLOC = 4                  # heads per core
NP = 2                    # head pairs per core
DLOC = HLOC * DK          # local output dims per core = 256
KT = D // 128             # contraction tiles = 8
NQC = S // 512            # query chunks = 4
NKB = S // 128            # key blocks = 16
NTT = S // 128            # token tiles = 16
SCALE = 1.0 / math.sqrt(DK)

_prog_cache = {}


def _build(masked: bool):
    import concourse.mybir as mybir
    import concourse.tile as tile
    from concourse import bacc

    f16 = mybir.dt.float16
    f32 = mybir.dt.float32
    EXP = mybir.ActivationFunctionType.Exp
    MUL = mybir.AluOpType.mult
    ADD = mybir.AluOpType.add

    nc = bacc.Bacc("TRN2", target_bir_lowering=False, debug=False)

    def din(name, shape, dt=f16):
        return nc.dram_tensor(name, shape, dt, kind="ExternalInput").ap()

    xq = din("xq", [D, S])             # query^T, this core's batch
    xk = din("xk", [D, S])
    xv = din("xv", [D, S])
    # projection weights prearranged: w_arr[p, kt, m] = W[hs+m, kt*128+p]
    wq = din("wq", [128, KT * DLOC])
    wk = din("wk", [128, KT * DLOC])
    wv = din("wv", [128, KT * DLOC])
    wo = din("wo", [DLOC, D])          # Wo[:, hs:hs+256].T
    bq = din("bq", [DLOC], f32)
    bk = din("bk", [DLOC], f32)
    bv = din("bv", [DLOC], f32)
    mb = din("mb", [S], f32)           # additive mask bias per key (0 / -1e30)
    out = nc.dram_tensor("out", [S, D], f16, kind="ExternalOutput").ap()

    with tile.TileContext(nc) as tc:
        with (
            tc.tile_pool(name="wsb", bufs=1) as wsb,
            tc.tile_pool(name="xsb", bufs=1) as xsb,
            tc.tile_pool(name="qkv", bufs=1) as qkv,
            tc.tile_pool(name="esb", bufs=6) as esb,
            tc.tile_pool(name="nrm", bufs=3) as nrm,
            tc.tile_pool(name="osb", bufs=4) as osb,
            tc.tile_pool(name="pj", bufs=2, space="PSUM") as pj,
            tc.tile_pool(name="psc", bufs=2, space="PSUM") as psc,
            tc.tile_pool(name="pcx", bufs=1, space="PSUM") as pcx,
        ):
            # ---- weight / bias / mask DMAs (small, front of queues) ----
            wk_sb = wsb.tile([128, KT, DLOC], f16, tag="wk")
            nc.gpsimd.dma_start(out=wk_sb, in_=wk.rearrange("p (kt m) -> p kt m", kt=KT))
            wq_sb = wsb.tile([128, KT, DLOC], f16, tag="wq")
            nc.scalar.dma_start(out=wq_sb, in_=wq.rearrange("p (kt m) -> p kt m", kt=KT))
            w_sb = {"wq": wq_sb, "wk": wk_sb}

            # ---- x tiles: one big tile per tensor, few large DMAs --------
            # only sync (HWDGE), scalar (HWDGE), gpsimd (SWDGE) can issue
            # DMA, and each queue serializes issue+completion (~2us fixed
            # per DMA), so the critical path wants FEW, LARGE transfers.
            xk_b = xsb.tile([128, KT, S], f16, tag="xk", name="xk_b")
            xq_b = xsb.tile([128, KT, S], f16, tag="xq", name="xq_b")
            xv_b = xsb.tile([128, KT, S], f16, tag="xv", name="xv_b")
            xk_t = [xk_b[:, kt, :] for kt in range(KT)]
            xq_t = [xq_b[:, kt, :] for kt in range(KT)]
            xv_t = [xv_b[:, kt, :] for kt in range(KT)]
            xkr = xk.rearrange("(kt p) s -> p kt s", p=128)
            xqr = xq.rearrange("(kt p) s -> p kt s", p=128)
            xvr = xv.rearrange("(kt p) s -> p kt s", p=128)

            # deadline-ordered waves; tiny bias loads ride the cheap SWDGE
            # (gpsimd) queue after the critical xk/wv transfers.
            nc.sync.dma_start(out=xk_b[:, 0:3, :], in_=xkr[:, 0:3, :])
            nc.scalar.dma_start(out=xk_b[:, 3:6, :], in_=xkr[:, 3:6, :])
            nc.gpsimd.dma_start(out=xk_b[:, 6:8, :], in_=xkr[:, 6:8, :])
            wv_sb = wsb.tile([128, KT, DLOC], f16, tag="wv")
            nc.gpsimd.dma_start(out=wv_sb, in_=wv.rearrange("p (kt m) -> p kt m", kt=KT))
            w_sb["wv"] = wv_sb
            nc.sync.dma_start(out=xq_b[:, 0:4, 0:512], in_=xqr[:, 0:4, 0:512])
            nc.scalar.dma_start(out=xq_b[:, 4:8, 0:512], in_=xqr[:, 4:8, 0:512])
            bq_sb = wsb.tile([128, NP], f32, tag="bq")
            nc.gpsimd.dma_start(out=bq_sb, in_=bq.rearrange("(pr i) -> i pr", pr=NP))
            bk_sb = wsb.tile([128, NP], f32, tag="bk")
            nc.gpsimd.dma_start(out=bk_sb, in_=bk.rearrange("(pr i) -> i pr", pr=NP))
            bv_row = wsb.tile([1, DLOC], f32, tag="bv_row")
            nc.gpsimd.dma_start(out=bv_row, in_=bv.unsqueeze(0))
            bv_bc = wsb.tile([128, DLOC], f32, tag="bv_bc")
            nc.gpsimd.partition_broadcast(bv_bc, bv_row)
            mb_sb = wsb.tile([128, NKB], f32, tag="mb")
            nc.gpsimd.dma_start(out=mb_sb, in_=mb.rearrange("(kb p) -> p kb", p=128))
            nc.gpsimd.dma_start(out=xv_b[:, :, 0:1024], in_=xvr[:, :, 0:1024])
            # later waves
            nc.sync.dma_start(out=xq_b[:, :, 512:1024], in_=xqr[:, :, 512:1024])
            nc.gpsimd.dma_start(out=xv_b[:, :, 1024:2048], in_=xvr[:, :, 1024:2048])
            nc.sync.dma_start(out=xq_b[:, :, 1024:2048], in_=xqr[:, :, 1024:2048])
            wo_sb = [wsb.tile([128, D], f16, tag=f"wo{p}", name=f"wo{p}")
                     for p in range(NP)]
            nc.gpsimd.dma_start(out=wo_sb[0], in_=wo[0:128, :])
            nc.gpsimd.dma_start(out=wo_sb[1], in_=wo[128:256, :])

            # ---- persistent per-pair activations -------------------------
            qT = [qkv.tile([128, S], f16, tag=f"qT{p}", name=f"qT{p}") for p in range(NP)]
            kT = [qkv.tile([128, S], f16, tag=f"kT{p}", name=f"kT{p}") for p in range(NP)]
            vv = [qkv.tile([128, 2, NKB, 68], f16, tag=f"vv{p}", name=f"vv{p}")
                  for p in range(NP)]
            for p in range(NP):
                nc.vector.memset(vv[p][:, :, :, 64:65], 1.0)
            ctxn = [qkv.tile([128, S], f16, tag=f"ctxn{p}", name=f"ctxn{p}")
                    for p in range(NP)]

            # ---- projections --------------------------------------------
            def proj_qk_kt_outer(p, which, qcs, bank_of):
                """kt-outer K/Q projection for chunks qcs of pair p, kt
                visited in expected DMA-arrival order."""
                w, bias, dst = (("wq", bq_sb, qT) if which == "q"
                                else ("wk", bk_sb, kT))
                for i, kt in enumerate([6, 7, 0, 1, 2, 3, 4, 5]):
                    for qc in qcs:
                        ps = bank_of[qc]
                        nc.tensor.matmul(ps, w_sb[w][:, kt, p * 128:(p + 1) * 128],
                                         xq_t[kt][:, qc * 512:(qc + 1) * 512]
                                         if which == "q" else
                                         xk_t[kt][:, qc * 512:(qc + 1) * 512],
                                         start=(i == 0), stop=(i == KT - 1))
                for qc in qcs:
                    sl = slice(qc * 512, (qc + 1) * 512)
                    nc.vector.tensor_scalar(out=dst[p][:, sl], in0=bank_of[qc],
                                            scalar1=bias[:, p:p + 1], scalar2=None,
                                            op0=ADD)

            def proj_qk_chunk(p, which, qc):
                """Single (pair, chunk) projection through the pj pool."""
                w, bias, dst, xt = (("wq", bq_sb, qT, xq_t) if which == "q"
                                    else ("wk", bk_sb, kT, xk_t))
                ps = pj.tile([128, 512], f32, tag="pj", name="ps")
                sl = slice(qc * 512, (qc + 1) * 512)
                for kt in range(KT):
                    nc.tensor.matmul(ps, w_sb[w][:, kt, p * 128:(p + 1) * 128],
                                     xt[kt][:, sl],
                                     start=(kt == 0), stop=(kt == KT - 1))
                nc.vector.tensor_scalar(out=dst[p][:, sl], in0=ps,
                                        scalar1=bias[:, p:p + 1], scalar2=None,
                                        op0=ADD)

            def proj_v_tt(tts):
                """V projection for token tiles tts, both pairs at once."""
                for tt in tts:
                    ps = pj.tile([128, 512], f32, tag="pj", name="ps")
                    sl = slice(tt * 128, (tt + 1) * 128)
                    for kt in range(KT):
                        nc.tensor.matmul(ps[:, 0:DLOC], xv_t[kt][:, sl],
                                         w_sb["wv"][:, kt, :],
                                         start=(kt == 0), stop=(kt == KT - 1))
                    for p in range(NP):
                        for h in range(2):
                            d0 = (p * 2 + h) * 64
                            nc.vector.tensor_tensor(
                                out=vv[p][:, h, tt, 0:64],
                                in0=ps[:, d0:d0 + 64],
                                in1=bv_bc[:, d0:d0 + 64], op=ADD)

            def outproj_tt(tts, store_eng=None):
                store_eng = store_eng or nc.sync
                for tt in tts:
                    tsl = slice(tt * 128, (tt + 1) * 128)
                    for eh in range(2):
                        po = pj.tile([128, 512], f32, tag="pj", name="po")
                        esl = slice(eh * 512, (eh + 1) * 512)
                        for p in range(NP):
                            nc.tensor.matmul(po, ctxn[p][:, tsl], wo_sb[p][:, esl],
                                             start=(p == 0), stop=(p == NP - 1))
                        oc = osb.tile([128, 512], f16, tag="oc", name="oc")
                        nc.vector.tensor_copy(oc, po)
                        store_eng.dma_start(out=out[tsl, esl], in_=oc)

            # ---- attention ----------------------------------------------
            # kb-granular pipeline: one psco tile [128, h0-q|h1-q] per key
            # block, ring of 2 => scores(kb) only waits on exp(kb-2), which
            # the ACT engine finished long ago. One exp ACTIVATE per kb
            # covers both heads (and the mask bias, when present).
            def attn_qc(p, qc, injects=()):
                injects = list(injects)
                qsl = slice(qc * 512, (qc + 1) * 512)
                pctx = [pcx.tile([65, 512], f32, tag=f"cx{h}", name=f"cx{h}")
                        for h in range(2)]

                def scores_kb(kb):
                    psco = psc.tile([128, 1024], f32, tag="sc", name="sc")
                    ksl = slice(kb * 128, (kb + 1) * 128)
                    for h in range(2):
                        hsl = slice(h * 64, (h + 1) * 64)
                        nc.tensor.matmul(
                            psco[:, h * 512:(h + 1) * 512],
                            kT[p][hsl, ksl], qT[p][hsl, qsl],
                            start=True, stop=True,
                            tile_position=(h * 64, 0))
                    return psco

                def exp_ctx_kb(kb, psco):
                    et = esb.tile([128, 1024], f16, tag="e", name="et")
                    if masked:
                        nc.scalar.activation(et, psco, EXP,
                                             bias=mb_sb[:, kb:kb + 1], scale=SCALE)
                    else:
                        nc.scalar.activation(et, psco, EXP, scale=SCALE)
                    for h in range(2):
                        nc.tensor.matmul(
                            pctx[h], vv[p][:, h, kb, 0:65],
                            et[:, h * 512:(h + 1) * 512],
                            start=(kb == 0), stop=(kb == NKB - 1))

                ring = [scores_kb(0), scores_kb(1)]
                for kb in range(2, NKB):
                    ring.append(scores_kb(kb))
                    if injects:
                        injects.pop(0)()
                    exp_ctx_kb(kb - 2, ring.pop(0))
                while injects:
                    injects.pop(0)()
                exp_ctx_kb(NKB - 2, ring.pop(0))
                exp_ctx_kb(NKB - 1, ring.pop(0))

                # evacuate pctx first (both heads) so the next unit's ctx
                # chain gets its PSUM banks back ~2.5us earlier; the
                # recip/broadcast/multiply then run off the SBUF copies.
                cds, cls = [], []
                for h in range(2):
                    cd = nrm.tile([64, 512], f32, tag=f"cd{h}", name="cd")
                    nc.vector.tensor_copy(cd, pctx[h][0:64, :])
                    cl = nrm.tile([1, 512], f32, tag=f"cl{h}", name="cl")
                    nc.vector.tensor_copy(cl, pctx[h][64:65, :])
                    cds.append(cd)
                    cls.append(cl)
                for h in range(2):
                    hsl = slice(h * 64, (h + 1) * 64)
                    rl = nrm.tile([1, 512], f32, tag="rl", name="rl")
                    nc.vector.reciprocal_approx_fast(rl, cls[h])
                    rl_bc = nrm.tile([64, 512], f32, tag="rlb", name="rlb")
                    nc.gpsimd.partition_broadcast(rl_bc, rl)
                    nc.vector.tensor_tensor(out=ctxn[p][hsl, qsl],
                                            in0=cds[h], in1=rl_bc, op=MUL)

            # ---- prologue: PE warmup junk, K-proj pair0, Q chunk0 -------
            ksc = psc.tile([128, 1024], f32, tag="sc", name="ksc")
            for wu in range(14):
                nc.tensor.matmul(ksc[:, 0:512], w_sb["wk"][:, wu % 8, :128],
                                 w_sb["wq"][:, (wu % 4) * 2:(wu % 4) * 2 + 2, :],
                                 start=True, stop=True)
            kpj = [pj.tile([128, 512], f32, tag="pj", name="kpj") for _ in range(2)]
            kbank = {0: ksc[:, 0:512], 1: ksc[:, 512:1024], 2: kpj[0], 3: kpj[1]}
            proj_qk_kt_outer(0, "k", [0, 1, 2, 3], kbank)
            for wu in range(6):
                nc.tensor.matmul(ksc[:, 0:512], w_sb["wk"][:, wu % 8, :128],
                                 w_sb["wq"][:, (wu % 4) * 2:(wu % 4) * 2 + 2, :],
                                 start=True, stop=True)
            proj_qk_chunk(0, "q", 0)

            # ---- attention schedule with injected projection work -------
            attn_qc(0, 0, [
                lambda: proj_v_tt([0, 1]),
                lambda: proj_v_tt([2, 3]),
                lambda: proj_v_tt([4, 5]),
                lambda: proj_v_tt([6, 7]),
                lambda: proj_v_tt([8, 9]),
                lambda: proj_v_tt([10, 11]),
                lambda: proj_v_tt([12, 13]),
                lambda: proj_v_tt([14, 15]),
                lambda: proj_qk_chunk(0, "q", 1),
                lambda: proj_qk_chunk(0, "q", 2),
            ])
            attn_qc(0, 1, [
                lambda: proj_qk_chunk(0, "q", 3),
                lambda: proj_qk_chunk(1, "k", 0),
                lambda: proj_qk_chunk(1, "k", 1),
                lambda: proj_qk_chunk(1, "k", 2),
                lambda: proj_qk_chunk(1, "k", 3),
            ])
            attn_qc(0, 2, [
                lambda: proj_qk_chunk(1, "q", 0),
                lambda: proj_qk_chunk(1, "q", 1),
                lambda: proj_qk_chunk(1, "q", 2),
                lambda: proj_qk_chunk(1, "q", 3),
            ])
            attn_qc(0, 3, [])
            attn_qc(1, 0, [])
            attn_qc(1, 1, [
                lambda: outproj_tt([0, 1]),
                lambda: outproj_tt([2, 3]),
            ])
            attn_qc(1, 2, [
                lambda: outproj_tt([4, 5]),
                lambda: outproj_tt([6, 7]),
            ])
            attn_qc(1, 3, [
                lambda: outproj_tt([8, 9]),
                lambda: outproj_tt([10, 11]),
            ])
            outproj_tt([12, 13], store_eng=nc.scalar)
            outproj_tt([14, 15], store_eng=nc.sync)

    nc.compile()
    return nc


def _get_prog(masked: bool):
    key = masked
    if key not in _prog_cache:
        _prog_cache[key] = _build(masked)
    return _prog_cache[key]


def make_in_maps(query, key, value, mask, Wq, bq, Wk, bk, Wv, bv, Wo, bo):
    query = np.asarray(query)
    key = np.asarray(key)
    value = np.asarray(value)
    mask = np.asarray(mask)
    Wq, bq = np.asarray(Wq), np.asarray(bq)
    Wk, bk = np.asarray(Wk), np.asarray(bk)
    Wv, bv = np.asarray(Wv), np.asarray(bv)
    Wo = np.asarray(Wo)

    def t16(x):  # [S, B, D] -> contiguous [D, B, S] fp16
        return np.ascontiguousarray(x.transpose(2, 1, 0).astype(np.float16))

    def warr(W, hs):  # [128, KT*DLOC]: row p = concat_kt W[hs+m, kt*128+p]
        wt = W[hs:hs + DLOC, :].T.astype(np.float16)       # [kt*128+p, m]
        return np.ascontiguousarray(
            wt.reshape(KT, 128, DLOC).transpose(1, 0, 2).reshape(128, KT * DLOC))

    xq3, xk3, xv3 = t16(query), t16(key), t16(value)
    xqb = [np.ascontiguousarray(xq3[:, b, :]) for b in range(B)]
    xkb = [np.ascontiguousarray(xk3[:, b, :]) for b in range(B)]
    xvb = [np.ascontiguousarray(xv3[:, b, :]) for b in range(B)]
    mbias = np.where(mask.reshape(S), 0.0, -1e30).astype(np.float32)

    wqs = [warr(Wq, g * DLOC) for g in range(4)]
    wks = [warr(Wk, g * DLOC) for g in range(4)]
    wvs = [warr(Wv, g * DLOC) for g in range(4)]
    wos = [np.ascontiguousarray(Wo[:, g * DLOC:(g + 1) * DLOC].T.astype(np.float16))
           for g in range(4)]

    in_maps = []
    for c in range(NCORES):
        b, g = c // 4, c % 4
        hs = g * DLOC
        in_maps.append({
            "xq": xqb[b], "xk": xkb[b], "xv": xvb[b],
            "wq": wqs[g], "wk": wks[g], "wv": wvs[g], "wo": wos[g],
            "bq": bq[hs:hs + DLOC].astype(np.float32),
            "bk": bk[hs:hs + DLOC].astype(np.float32),
            "bv": bv[hs:hs + DLOC].astype(np.float32),
            "mb": mbias,
        })
    return in_maps


def kernel(query, key, value, mask, Wq, bq, Wk, bk, Wv, bv, Wo, bo):
    from concourse.bass_utils import run_bass_kernel_spmd

    mask = np.asarray(mask)
    bo = np.asarray(bo)
    masked = not bool(mask.all())
    nc = _get_prog(masked)
    in_maps = make_in_maps(query, key, value, mask, Wq, bq, Wk, bk, Wv, bv, Wo, bo)

    res = run_bass_kernel_spmd(nc, in_maps, core_ids=list(range(NCORES)))
    acc = np.zeros((S, B, D), dtype=np.float64)
    for c in range(NCORES):
        acc[:, c // 4, :] += res.results[c]["out"].astype(np.float64)
    acc += bo.astype(np.float64)
    return acc.astype(np.float32)
